# revision 2
# baseline (speedup 1.0000x reference)
"""PartitionPadding kernel for Trainium2 (8 NeuronCores, Bass).

Problem: atom_features [N=1e6, D=128] f32, atom_partition_indices [N] int64
(sorted, values in [0, 50)). Output: padded [50, 21000, 128] f32 where
partition b's rows are placed at [b, 0:counts[b], :], zeros elsewhere
(then reordered by the reference's "keep nonempty" rule, identity in
practice).

Because the indices are sorted, each partition's atoms form a contiguous
slab of the input, so the whole op is a ragged->padded memcpy. The host
computes the 50 partition counts (cheap metadata); the device does all
bulk data movement.

Sharding: the 1,050,000 output rows are split into 8 equal contiguous
windows of 131,250 rows, one per core. Each core receives the contiguous
input-row window it needs (a zero-copy slice) and runs the same SPMD
program: it branches on partition_id() into its own static list of ~7-8
large contiguous DRAM->DRAM DMA copies (HWDGE, ~10 MB each). Pad regions
are never written: run_bass_kernel_spmd pre-zeroes ExternalOutput buffers
(donated zero buffers under the axon/PJRT path), so unwritten rows read
back as zeros.
"""

import numpy as np

BATCH = 50
MAX_ATOMS = 21000
D = 128
N_CORES = 8
TOT_ROWS = BATCH * MAX_ATOMS          # 1,050,000
ROWS_PER_CORE = TOT_ROWS // N_CORES   # 131,250


def _plan(counts, n_rows):
    """Per-core copy segments.

    Returns (core_segs, core_src_base, r_in) where core_segs[k] is a list of
    (src_local_row, dst_local_row, n_rows) for core k, relative to its input
    window [core_src_base[k], core_src_base[k]+r_in) and output window
    [k*ROWS_PER_CORE, (k+1)*ROWS_PER_CORE).
    """
    counts = np.asarray(counts, dtype=np.int64)
    starts = np.cumsum(counts) - counts
    r_in = min(ROWS_PER_CORE, n_rows)
    core_segs, core_src_base = [], []
    for k in range(N_CORES):
        o0, o1 = k * ROWS_PER_CORE, (k + 1) * ROWS_PER_CORE
        src_ranges = []  # (src_abs_start, src_abs_end, dst_abs_start)
        for b in range(o0 // MAX_ATOMS, (o1 - 1) // MAX_ATOMS + 1):
            base = b * MAX_ATOMS
            lo = max(o0, base) - base          # local-in-partition row range
            hi = min(o1, base + MAX_ATOMS) - base
            hi = min(hi, int(counts[b]))        # clip to the data region
            if hi > lo:
                src_ranges.append((int(starts[b]) + lo, int(starts[b]) + hi,
                                   base + lo))
        if src_ranges:
            span = src_ranges[-1][1] - src_ranges[0][0]
            assert span <= r_in, (k, span, r_in)
            src_base = max(0, min(src_ranges[0][0], n_rows - r_in))
        else:
            src_base = 0
        segs = []
        for (s0, s1, d) in src_ranges:
            a, c, n = s0 - src_base, d - o0, s1 - s0
            # walrus splits each DMA's flat element count across 8 engines
            # and must factor the per-engine count into 16-bit ISA fields;
            # awkward row counts (e.g. prime) fail codegen. Row counts
            # divisible by 16 always factor (inner dim 128), so emit a
            # 16-aligned bulk DMA plus a <=15-row tail DMA.
            bulk = n - (n % 16)
            if bulk:
                segs.append((a, c, bulk))
            if n % 16:
                segs.append((a + bulk, c + bulk, n % 16))
        core_segs.append(segs)
        core_src_base.append(src_base)
    return core_segs, core_src_base, r_in


def _build_nc(core_segs, r_in, repeat=1):
    """SPMD Bass program: branch on partition id, run that core's static DMAs."""
    import concourse.bass as bass
    import concourse.mybir as mybir

    nc = bass.Bass()
    x = nc.dram_tensor("x", [r_in, D], mybir.dt.float32, kind="ExternalInput")
    y = nc.dram_tensor("y", [ROWS_PER_CORE, D], mybir.dt.float32,
                       kind="ExternalOutput")
    with nc.Block() as block, nc.semaphore("dma_sem") as sem:

        @block.sync
        def _(g):
            pid = g.partition_id()
            for k in range(len(core_segs)):
                with g.If(pid == k):
                    tot = 0
                    for _ in range(repeat):
                        for (a, c, n) in core_segs[k]:
                            g.dma_start(out=y[c:c + n, :],
                                        in_=x[a:a + n, :]).then_inc(sem, 16)
                            tot += 16
                        g.wait_ge(sem, tot)
    return nc


def _make_in_maps(atom_features, core_src_base, r_in):
    return [{"x": atom_features[b:b + r_in]} for b in core_src_base]


def kernel(atom_features, atom_partition_indices):
    from concourse.bass_utils import run_bass_kernel_spmd

    atom_features = np.ascontiguousarray(np.asarray(atom_features,
                                                    dtype=np.float32))
    idx = np.asarray(atom_partition_indices).astype(np.int64, copy=False)
    n_rows = atom_features.shape[0]
    counts = np.bincount(idx, minlength=BATCH)
    assert counts.sum() == n_rows

    core_segs, core_src_base, r_in = _plan(counts, n_rows)
    nc = _build_nc(core_segs, r_in)
    in_maps = _make_in_maps(atom_features, core_src_base, r_in)
    res = run_bass_kernel_spmd(nc, in_maps, list(range(N_CORES)))
    out = np.concatenate([res.results[k]["y"] for k in range(N_CORES)], axis=0)
    padded = out.reshape(BATCH, MAX_ATOMS, D)

    # Reference keeps nonempty examples (sum != 0), reordered to the front
    # with fill 0. With every partition nonempty this is the identity.
    mask = counts > 0
    if not mask.all():
        keep = np.nonzero(mask)[0]
        keep = np.concatenate(
            [keep, np.zeros(BATCH - keep.size, dtype=np.int64)])
        padded = padded[keep]
    return padded
